# revision 1
# baseline (speedup 1.0000x reference)
"""Trainium2 Bass kernel for nn_CIN: 3-layer compressed-interaction network.

Reference computation (per layer l with kernel k_l [O,H,M]):
    x_{l+1}[b,o,d] = sum_{h,m} x_l[b,h,d] * x0[b,m,d] * k_l[o,h,m]
    out = concat_l(sum_d x_{l+1}[b,o,d])          # (B, 3*128)

Sharding: pure data-parallel over B across 8 cores (512 batch each).

Per-core algorithm (bd = 512*16 = 8192 columns, 64 chunks of 128):
  L0/L1 ("orientation A"): for each bd-chunk c, use x_l[:, c] as the PE
  stationary operand and stream the host-pretransposed kernel
  k_l_p[h, (m,o)]; PSUM gets t'_m[bd, o] = sum_h x[h,bd]*k[o,h,m] for 4
  m's per matmul group.  Matmuls run in split precision: x and K are
  each decomposed hi+lo in bf16 and three bf16 matmuls (hh, hl, lh)
  accumulate in PSUM -- ~2x faster than fp32's 4 cyc/row at ~1e-5
  relative accuracy.  The data-dependent m-contraction
  x_{l+1}T[bd,o] = sum_m x0'[bd,m]*t'_m[bd,o] is split three ways:
  a DVE chain of fused scalar_tensor_tensor ops, plus an ACT
  (scale-copy) -> GPSIMD (add) chain, merged on GPSIMD.  L0's output is
  PE-transposed back to [h, bd] (and split hi/lo) for L1.  L1's output
  stays transposed (x2T) -- exactly what L2 needs.

  L2 (d-sum folded into PE): out3[b,o] = sum_{hm} k2[o,h,m] * w[b,h,m]
  with w[b,h,m] = sum_d x2[b,h,d]*x0[b,m,d].  Per chunk, one matmul
  lhsT=x2T_c[bd,h], rhs=X0E[bd,(m,b')] where X0E = x0'[bd,m]*E[bd,b']
  (E = batch indicator) contracts bd, yielding w[h,(m,b')] plus, via an
  appended plain-indicator block, the d-sum of x2 (=out2).  A final
  40-matmul PSUM accumulation k2p_m @ w_m produces out3[o,b].
"""

import numpy as np
from contextlib import ExitStack

import ml_dtypes
import concourse.bass as bass
import concourse.tile as tile
import concourse.mybir as mybir

F32 = mybir.dt.float32
BF16 = mybir.dt.bfloat16
ALU = mybir.AluOpType
AFT = mybir.ActivationFunctionType

B, M, D, O = 4096, 40, 16, 128
N_CORES = 8
BC = B // N_CORES          # 512 batch rows per core
MPG = 4                    # m's packed per matmul (4*128 = 512 free cols)
GROUPS = M // MPG          # 10
M1 = M + 1                 # x0t carries a trailing ones-column

# step-B scheduling per group g (of GROUPS):
#   "direct": DVE fused scalar_tensor_tensor straight from PSUM
#   "bulk":   ACT bulk-copies the group PSUM->SBUF, DVE fused ops on SBUF
#   "actgp":  per-m ACT scaled-copy (PSUM->SBUF) + GPSIMD tensor add
G_MODE = ["direct"] * 4 + ["bulk"] * 4 + ["actgp"] * 2


_ns_ctr = [0]


def _split_excess_waits(nc, max_waits=1):
    """walrus in this env rejects >1 sync-wait on one instruction (CTRL
    struct): move excess waits onto same-engine NoOps inserted before."""
    for f in nc.m.functions:
        for bb in f.blocks:
            new_list = []
            for inst in bb.instructions:
                si = inst.sync_info
                waits = list(si.on_wait) if si and si.on_wait else []
                if len(waits) > max_waits:
                    excess = waits[:-max_waits]
                    keep = waits[-max_waits:]
                    for i in range(0, len(excess), max_waits):
                        chunk = excess[i:i + max_waits]
                        _ns_ctr[0] += 1
                        nop = mybir.InstNoOp(
                            name=f"waitsplit-{_ns_ctr[0]}", ins=[], outs=[],
                            engine=inst.engine,
                            sync_info=mybir.SyncInfo(on_wait=chunk, on_update=[]),
                        )
                        nc.register_instruction(nop)
                        new_list.append(nop)
                    si.on_wait = keep
                    inst.sync_info = si
                new_list.append(inst)
            bb.instructions[:] = new_list


def build(n_chunks):
    """Build the per-core Bass program for n_chunks*128 bd columns."""
    bd = n_chunks * 128
    bcl = bd // D              # local batch count
    nb = (bcl + 127) // 128    # output b-tiles
    nc = bass.Bass("TRN2", target_bir_lowering=False, debug=False, num_devices=1)

    x0p2h_d = nc.dram_tensor("x0p2h", [128, bd], BF16, kind="ExternalInput")
    x0p2l_d = nc.dram_tensor("x0p2l", [128, bd], BF16, kind="ExternalInput")
    x0t_d = nc.dram_tensor("x0t", [bd, M1], F32, kind="ExternalInput")
    k0p2h_d = nc.dram_tensor("k0p2h", [128, M * O // 2], BF16,
                             kind="ExternalInput")
    k0p2l_d = nc.dram_tensor("k0p2l", [128, M * O // 2], BF16,
                             kind="ExternalInput")
    k1ph_d = nc.dram_tensor("k1ph", [O, M * O], BF16, kind="ExternalInput")
    k1pl_d = nc.dram_tensor("k1pl", [O, M * O], BF16, kind="ExternalInput")
    k2p_d = nc.dram_tensor("k2p", [O, M * O], F32, kind="ExternalInput")
    e41_d = nc.dram_tensor("e41", [128, M1 * 8], F32, kind="ExternalInput")
    iden_d = nc.dram_tensor("iden", [128, 128], F32, kind="ExternalInput")
    out_d = nc.dram_tensor("out", [bcl, 3 * O], F32, kind="ExternalOutput")

    with tile.TileContext(nc) as tc:
        with ExitStack() as perm:
            pp = perm.enter_context(tc.tile_pool(name="perm", bufs=1))
            x0t_sb = pp.tile([128, n_chunks * M1], F32, name="x0t_sb")
            nc.sync.dma_start(
                x0t_sb[:].rearrange("p (c m) -> p c m", m=M1),
                x0t_d.ap().rearrange("(c p) m -> p c m", p=128))
            e41_sb = pp.tile([128, M1 * 8], F32, name="e41_sb")
            nc.sync.dma_start(e41_sb[:], e41_d.ap())
            iden_sb = pp.tile([128, 128], F32, name="iden_sb")
            nc.sync.dma_start(iden_sb[:], iden_d.ap())
            k1ph_sb = pp.tile([O, M * O], BF16, name="k1ph_sb")
            nc.sync.dma_start(k1ph_sb[:], k1ph_d.ap())
            k1pl_sb = pp.tile([O, M * O], BF16, name="k1pl_sb")
            nc.sync.dma_start(k1pl_sb[:], k1pl_d.ap())
            x2T_sb = pp.tile([128, bd], F32, name="x2T_sb")
            o1_st = pp.tile([128, bcl], F32, name="o1_st")
            o2_st = pp.tile([128, bcl], F32, name="o2_st")
            o3_st = pp.tile([128, bcl], F32, name="o3_st")

            def sc_ap(c, m):
                return x0t_sb[:, c * M1 + m: c * M1 + m + 1]

            def mm3(pt, lh, ll, kh_g, kl_g, tp=None):
                """hi/lo split matmul group accumulating into PSUM pt."""
                nc.tensor.matmul(pt, lh, kh_g, start=True, stop=False,
                                 tile_position=tp)
                nc.tensor.matmul(pt, lh, kl_g, start=False, stop=False,
                                 tile_position=tp)
                nc.tensor.matmul(pt, ll, kh_g, start=False, stop=True,
                                 tile_position=tp)

            class StepB:
                """3-way split m-contraction consuming t'-groups.

                DVE chain (acc_d) eats the first DIRECT_G groups straight
                from PSUM via fused scalar_tensor_tensor; later groups are
                bulk-copied PSUM->SBUF by ACT and consumed by SBUF-side
                fused ops split between DVE and GPSIMD (acc_g), then
                merged on GPSIMD into merge_out."""

                def __init__(self, c, accs_d, acc_g, merge_out, tg_pool):
                    self.c, self.accs_d, self.acc_g = c, accs_d, acc_g
                    self.merge_out, self.tg_pool = merge_out, tg_pool
                    self.first_d = [True] * len(accs_d)
                    self.di = 0
                    self.first_g = True

                def _dve(self, src, sc):
                    i = self.di
                    self.di = (i + 1) % len(self.accs_d)
                    if self.first_d[i]:
                        nc.vector.tensor_scalar(
                            self.accs_d[i], src, sc, None, ALU.mult)
                        self.first_d[i] = False
                    else:
                        nc.vector.scalar_tensor_tensor(
                            self.accs_d[i], src, sc, self.accs_d[i],
                            ALU.mult, ALU.add)

                def eat(self, g, pt):
                    c = self.c
                    mode = G_MODE[g]
                    if mode == "direct":
                        for j in range(MPG):
                            self._dve(pt[:, j * O:(j + 1) * O],
                                      sc_ap(c, g * MPG + j))
                    elif mode == "bulk":
                        tg = self.tg_pool.tile([128, MPG * O], F32,
                                               name="tg", tag="tg")
                        nc.scalar.copy(tg[:], pt[:])
                        for j in range(MPG):
                            self._dve(tg[:, j * O:(j + 1) * O],
                                      sc_ap(c, g * MPG + j))
                    else:  # actgp
                        for j in range(MPG):
                            sc = sc_ap(c, g * MPG + j)
                            src = pt[:, j * O:(j + 1) * O]
                            if self.first_g:
                                nc.scalar.activation(
                                    self.acc_g, src, AFT.Copy, scale=sc)
                                self.first_g = False
                            else:
                                tg = self.tg_pool.tile([128, O], F32,
                                                       name="tgs", tag="tgs")
                                nc.scalar.activation(
                                    tg[:], src, AFT.Copy, scale=sc)
                                nc.gpsimd.tensor_tensor(
                                    self.acc_g, self.acc_g, tg[:], ALU.add)

                def finish(self):
                    nc.vector.tensor_tensor(
                        self.accs_d[0], self.accs_d[0], self.accs_d[1],
                        ALU.add)
                    nc.gpsimd.tensor_tensor(
                        self.merge_out, self.accs_d[0], self.acc_g, ALU.add)

            with ExitStack() as phA:
                pa = phA.enter_context(tc.tile_pool(name="phA", bufs=1))
                x0p2h_sb = pa.tile([128, bd], BF16, name="x0p2h_sb")
                nc.sync.dma_start(x0p2h_sb[:], x0p2h_d.ap())
                x0p2l_sb = pa.tile([128, bd], BF16, name="x0p2l_sb")
                nc.sync.dma_start(x0p2l_sb[:], x0p2l_d.ap())
                k0p2h_sb = pa.tile([128, M * O // 2], BF16, name="k0p2h_sb")
                nc.sync.dma_start(k0p2h_sb[:], k0p2h_d.ap())
                k0p2l_sb = pa.tile([128, M * O // 2], BF16, name="k0p2l_sb")
                nc.sync.dma_start(k0p2l_sb[:], k0p2l_d.ap())
                x1h_sb = pa.tile([128, bd], BF16, name="x1h_sb")
                x1l_sb = pa.tile([128, bd], BF16, name="x1l_sb")
                acc_pool = phA.enter_context(tc.tile_pool(name="accs", bufs=4))
                tg_pool = phA.enter_context(tc.tile_pool(name="tgs", bufs=4))
                pt_pool = phA.enter_context(
                    tc.tile_pool(name="ptp", bufs=4, space="PSUM"))
                ptr_pool = phA.enter_context(
                    tc.tile_pool(name="ptrp", bufs=2, space="PSUM"))

                # L0 (row-packed 2x: strips at partitions 0-39 and 64-103)
                # -> merge -> PE transpose -> split-cast to x1h/x1l
                for c in range(n_chunks):
                    cs = slice(c * 128, (c + 1) * 128)
                    acc_d = acc_pool.tile([128, 128], F32, name="acc_d",
                                          tag="acc_d")
                    acc_d2 = acc_pool.tile([128, 128], F32, name="acc_d2",
                                           tag="acc_d2")
                    acc_g = acc_pool.tile([128, 128], F32, name="acc_g",
                                          tag="acc_g")
                    accm = acc_pool.tile([128, 128], F32, name="accm",
                                         tag="accm")
                    sb = StepB(c, [acc_d[:], acc_d2[:]], acc_g[:], accm[:],
                               tg_pool)
                    for p in range(GROUPS // 2):
                        pt_a = pt_pool.tile([128, MPG * O], F32, name="pt",
                                            tag="pt")
                        pt_b = pt_pool.tile([128, MPG * O], F32, name="pt2",
                                            tag="pt")
                        ps = slice(p * MPG * O, (p + 1) * MPG * O)
                        la_h = x0p2h_sb[0:M, cs]
                        la_l = x0p2l_sb[0:M, cs]
                        lb_h = x0p2h_sb[64:64 + M, cs]
                        lb_l = x0p2l_sb[64:64 + M, cs]
                        nc.tensor.matmul(pt_a[:], la_h, k0p2h_sb[0:M, ps],
                                         start=True, stop=False)
                        nc.tensor.matmul(pt_b[:], lb_h,
                                         k0p2h_sb[64:64 + M, ps],
                                         start=True, stop=False)
                        nc.tensor.matmul(pt_a[:], la_h, k0p2l_sb[0:M, ps],
                                         start=False, stop=False)
                        nc.tensor.matmul(pt_b[:], lb_h,
                                         k0p2l_sb[64:64 + M, ps],
                                         start=False, stop=False)
                        nc.tensor.matmul(pt_a[:], la_l, k0p2h_sb[0:M, ps],
                                         start=False, stop=True)
                        nc.tensor.matmul(pt_b[:], lb_l,
                                         k0p2h_sb[64:64 + M, ps],
                                         start=False, stop=True)
                        sb.eat(2 * p, pt_a)
                        sb.eat(2 * p + 1, pt_b)
                    sb.finish()
                    ptr = ptr_pool.tile([128, 128], F32, name="ptr", tag="ptr")
                    nc.tensor.transpose(ptr[:], accm[:], iden_sb[:])
                    xh = x1h_sb[:, cs]
                    nc.scalar.copy(xh, ptr[:])
                    nc.vector.tensor_tensor(
                        x1l_sb[:, cs], ptr[:], xh, ALU.subtract)

                # L1: merge straight into resident x2T slices
                for c in range(n_chunks):
                    cs = slice(c * 128, (c + 1) * 128)
                    acc_d = acc_pool.tile([128, 128], F32, name="acc_d",
                                          tag="acc_d")
                    acc_d2 = acc_pool.tile([128, 128], F32, name="acc_d2",
                                           tag="acc_d2")
                    acc_g = acc_pool.tile([128, 128], F32, name="acc_g",
                                          tag="acc_g")
                    sb = StepB(c, [acc_d[:], acc_d2[:]], acc_g[:],
                               x2T_sb[:, cs], tg_pool)
                    for g in range(GROUPS):
                        pt = pt_pool.tile([128, MPG * O], F32, name="pt",
                                          tag="pt")
                        gs = slice(g * MPG * O, (g + 1) * MPG * O)
                        mm3(pt[:], x1h_sb[:, cs], x1l_sb[:, cs],
                            k1ph_sb[:, gs], k1pl_sb[:, gs])
                        sb.eat(g, pt)
                    sb.finish()

                # out1 = sum_d x1 = sum_d (x1h + x1l), strided reduces
                r1h = pa.tile([128, bcl], F32, name="r1h")
                r1l = pa.tile([128, bcl], F32, name="r1l")
                nc.vector.tensor_reduce(
                    r1h[:], x1h_sb[:].rearrange("p (b d) -> p b d", d=D),
                    mybir.AxisListType.X, ALU.add)
                nc.vector.tensor_reduce(
                    r1l[:], x1l_sb[:].rearrange("p (b d) -> p b d", d=D),
                    mybir.AxisListType.X, ALU.add)
                nc.vector.tensor_tensor(o1_st[:], r1h[:], r1l[:], ALU.add)

            # ---- phase B: L2 via indicator matmuls ----
            with ExitStack() as phB:
                pb = phB.enter_context(tc.tile_pool(name="phB", bufs=1))
                k2p_sb = pb.tile([O, M * O], F32, name="k2p_sb")
                nc.sync.dma_start(k2p_sb[:], k2p_d.ap())
                w_sb = pb.tile([128, M * bcl], F32, name="w_sb")
                x0e_pool = phB.enter_context(tc.tile_pool(name="x0es", bufs=3))
                pw_pool = phB.enter_context(
                    tc.tile_pool(name="pwp", bufs=3, space="PSUM"))
                po3_pool = phB.enter_context(
                    tc.tile_pool(name="po3p", bufs=1, space="PSUM"))
                ptp_pool = phB.enter_context(
                    tc.tile_pool(name="ptpp", bufs=2, space="PSUM"))

                e41_3d = e41_sb[:].rearrange("p (m e) -> p m e", e=8)
                w_4d = w_sb[:].rearrange("p (m b) -> p m b", b=bcl)
                for c in range(n_chunks):
                    x0e = x0e_pool.tile([128, M1 * 8], F32, name="x0e",
                                        tag="x0e")
                    nc.vector.tensor_tensor(
                        x0e[:].rearrange("p (m e) -> p m e", e=8),
                        x0t_sb[:, c * M1:(c + 1) * M1].unsqueeze(2)
                        .broadcast_to([128, M1, 8]),
                        e41_3d, ALU.mult)
                    pw = pw_pool.tile([128, M1 * 8], F32, name="pw", tag="pw")
                    nc.tensor.matmul(
                        pw[:], x2T_sb[:, c * 128:(c + 1) * 128],
                        x0e[:], start=True, stop=True)
                    # scatter w part: w[h, m, c*8+j] <- pw[h, m*8+j]
                    nc.scalar.copy(
                        w_4d[:, :, c * 8:(c + 1) * 8],
                        pw[:, 0:M * 8].rearrange("p (m e) -> p m e", e=8))
                    # out2 block: columns [320:328)
                    nc.scalar.copy(o2_st[:, c * 8:(c + 1) * 8],
                                   pw[:, M * 8:M1 * 8])

                po3 = po3_pool.tile([128, bcl], F32, name="po3")
                for m in range(M):
                    nc.tensor.matmul(
                        po3[:], k2p_sb[:, m * O:(m + 1) * O],
                        w_sb[:, m * bcl:(m + 1) * bcl],
                        start=(m == 0), stop=(m == M - 1))
                nc.scalar.copy(o3_st[:], po3[:])

                # ---- outputs: transpose [o, b] tiles to [b, o] and store
                tb_pool = phB.enter_context(tc.tile_pool(name="tbs", bufs=3))
                for l, st in enumerate((o1_st, o2_st, o3_st)):
                    for j in range(nb):
                        tw = min(128, bcl - j * 128)
                        ptp = ptp_pool.tile([128, 128], F32, name="ptp",
                                            tag="ptp")
                        nc.tensor.transpose(
                            ptp[0:tw, :], st[:, j * 128:j * 128 + tw],
                            iden_sb[:])
                        tb = tb_pool.tile([128, 128], F32, name="tb", tag="tb")
                        nc.scalar.copy(tb[0:tw, :], ptp[0:tw, :])
                        nc.sync.dma_start(
                            out_d.ap()[j * 128:j * 128 + tw,
                                       l * O:(l + 1) * O],
                            tb[0:tw, :])

    _split_excess_waits(nc)
    return nc


def _bf16_split(a):
    hi = a.astype(ml_dtypes.bfloat16)
    lo = (a - hi.astype(np.float32)).astype(ml_dtypes.bfloat16)
    return hi, lo


def host_prep(x0c, k0, k1, k2):
    """Per-core input prep. x0c: (bcl, M, D) float32."""
    bcl = x0c.shape[0]
    x0m = np.ascontiguousarray(
        x0c.transpose(1, 0, 2).reshape(M, bcl * D), dtype=np.float32)
    x0mh, x0ml = _bf16_split(x0m)
    x0p2h = np.zeros((128, bcl * D), ml_dtypes.bfloat16)
    x0p2l = np.zeros((128, bcl * D), ml_dtypes.bfloat16)
    x0p2h[0:M] = x0mh
    x0p2h[64:64 + M] = x0mh
    x0p2l[0:M] = x0ml
    x0p2l[64:64 + M] = x0ml
    x0t = np.concatenate(
        [x0c.transpose(0, 2, 1).reshape(bcl * D, M),
         np.ones((bcl * D, 1), np.float32)], axis=1)
    x0t = np.ascontiguousarray(x0t, dtype=np.float32)
    k0p = np.ascontiguousarray(
        k0.transpose(1, 2, 0).reshape(M, M * O), dtype=np.float32)
    k0ph, k0pl = _bf16_split(k0p)
    npair = GROUPS // 2
    k0p2h = np.zeros((128, M * O // 2), ml_dtypes.bfloat16)
    k0p2l = np.zeros((128, M * O // 2), ml_dtypes.bfloat16)
    for p in range(npair):
        ps = slice(p * MPG * O, (p + 1) * MPG * O)
        k0p2h[0:M, ps] = k0ph[:, (2 * p) * MPG * O:(2 * p + 1) * MPG * O]
        k0p2h[64:64 + M, ps] = k0ph[:, (2 * p + 1) * MPG * O:
                                    (2 * p + 2) * MPG * O]
        k0p2l[0:M, ps] = k0pl[:, (2 * p) * MPG * O:(2 * p + 1) * MPG * O]
        k0p2l[64:64 + M, ps] = k0pl[:, (2 * p + 1) * MPG * O:
                                    (2 * p + 2) * MPG * O]
    k1p = np.ascontiguousarray(
        k1.transpose(1, 2, 0).reshape(O, M * O), dtype=np.float32)
    k1ph, k1pl = _bf16_split(k1p)
    k2p = np.ascontiguousarray(
        k2.transpose(1, 2, 0).reshape(O, M * O), dtype=np.float32)
    e8 = (np.arange(128)[:, None] // D == np.arange(8)[None, :])
    e8 = e8.astype(np.float32)
    e41 = np.ascontiguousarray(
        np.tile(e8[:, None, :], (1, M1, 1)).reshape(128, M1 * 8))
    iden = np.eye(128, dtype=np.float32)
    return {"x0p2h": x0p2h, "x0p2l": x0p2l, "x0t": x0t,
            "k0p2h": k0p2h, "k0p2l": k0p2l, "k1ph": k1ph, "k1pl": k1pl,
            "k2p": k2p, "e41": e41, "iden": iden}


_nc_cache = {}


def _get_nc(n_chunks):
    if n_chunks not in _nc_cache:
        _nc_cache[n_chunks] = build(n_chunks)
    return _nc_cache[n_chunks]


def kernel(x0, k0, k1, k2):
    from concourse.bass_utils import run_bass_kernel_spmd
    x0 = np.asarray(x0, dtype=np.float32)
    k0 = np.asarray(k0, dtype=np.float32)
    k1 = np.asarray(k1, dtype=np.float32)
    k2 = np.asarray(k2, dtype=np.float32)
    n_chunks = (BC * D) // 128
    nc = _get_nc(n_chunks)
    in_maps = [host_prep(x0[c * BC:(c + 1) * BC], k0, k1, k2)
               for c in range(N_CORES)]
    res = run_bass_kernel_spmd(nc, in_maps, core_ids=list(range(N_CORES)))
    out = np.concatenate([r["out"] for r in res.results], axis=0)
    return out.astype(np.float32)

